# revision 20
# baseline (speedup 1.0000x reference)
"""AttnMPNN (GNN message passing w/ edge softmax) on 8 Trainium2 NeuronCores.

Strategy (graph-partition by destination node):
  - 8 cores each own a contiguous range of NPC = N/8 destination nodes.
  - Host-side sharding: edges are bucketed by (owner core, dst slot of 128
    nodes, src half) and padded to a uniform static grid so one SPMD program
    serves all cores. All DMA gather indices fit int16 by rebasing:
    src is split in two table halves, dst is core-local.
  - Device per core:
      A = nf @ W_attn[:64] + b_attn  (all nodes, DRAM scratch)
      B = nf_own @ W_attn[64:]       (own nodes, DRAM scratch)
      per dst-slot g (128 nodes): gather A[src], nf[src], B[dst] rows via
      dma_gather; h = relu(A[src]+B[dst]); logit = h . w_fc; lrelu; w = exp;
      segment-reduce messages with selection-matrix matmuls accumulated in
      PSUM over the slot (psum[p,:64] = sum_e w_e*nf[src_e] for dst%128==p,
      psum[p,64] = sum_e w_e); then agg_m = psum[:, :64]/wsum/deg and
      out = [nf_own | agg_m] @ W_node + b_node for the slot's 128 nodes.
  - Softmax max-subtraction is dropped (mathematically invariant; logits are
    O(1) so exp is safe in f32). Degree counts come from the same host-side
    index metadata that drives the sharding (bincount of dst).
"""

import numpy as np

P = 128
D = 64


def _ceil_div(a, b):
    return (a + b - 1) // b


def _wrap16(arr2d):
    """[S, n] int16 -> [S, 128, n//16] wrapped by 16, replicated to 8 groups."""
    S, n = arr2d.shape
    w = arr2d.reshape(S, n // 16, 16).transpose(0, 2, 1)  # [S,16,n/16]
    return np.ascontiguousarray(np.tile(w, (1, 8, 1)))


def _build_program(cfg, stage):
    import concourse.bass as bass
    import concourse.tile as tile
    from concourse import bacc, mybir
    from concourse.masks import make_identity

    NF_PAD = cfg["NF_PAD"]
    HALF = cfg["HALF"]
    NPAD = cfg["NPAD"]
    SLOTS = NPAD // P
    B0 = cfg["B0"]
    B1 = cfg["B1"]
    BPS = B0 + B1
    f32 = mybir.dt.float32
    i16 = mybir.dt.int16

    nc = bacc.Bacc("TRN2", target_bir_lowering=False, debug=False,
                   enable_asserts=False)

    t_nf = nc.dram_tensor("nf", (NF_PAD, D), f32, kind="ExternalInput")
    t_nfo = nc.dram_tensor("nfo", (NPAD, D), f32, kind="ExternalInput")
    t_w1 = nc.dram_tensor("w1", (D, D), f32, kind="ExternalInput")
    t_w2 = nc.dram_tensor("w2", (D, D), f32, kind="ExternalInput")
    t_ba = nc.dram_tensor("ba", (D,), f32, kind="ExternalInput")
    t_wfc = nc.dram_tensor("wfc", (D,), f32, kind="ExternalInput")
    t_wn = nc.dram_tensor("wn", (2 * D, D), f32, kind="ExternalInput")
    t_bn = nc.dram_tensor("bn", (D,), f32, kind="ExternalInput")
    t_is0 = nc.dram_tensor("is0", (SLOTS, P, B0 * 8), i16, kind="ExternalInput")
    t_is1 = nc.dram_tensor("is1", (SLOTS, P, B1 * 8), i16, kind="ExternalInput")
    t_idst = nc.dram_tensor("idst", (SLOTS, P, BPS * 8), i16, kind="ExternalInput")
    t_pm = nc.dram_tensor("pm", (SLOTS, P, BPS), f32, kind="ExternalInput")
    t_pmT = nc.dram_tensor("pmT", (SLOTS, BPS * P), f32, kind="ExternalInput")
    t_dginv = nc.dram_tensor("dginv", (P, SLOTS), f32, kind="ExternalInput")
    t_out = nc.dram_tensor("out", (NPAD, D), f32, kind="ExternalOutput")

    bf16 = mybir.dt.bfloat16
    ck = "ExternalOutput" if stage == 1 else "ExternalInput"
    t_C = nc.dram_tensor("C_scr", (NF_PAD, 2 * D), bf16, kind=ck)
    t_B = nc.dram_tensor("B_scr", (NPAD, D), bf16, kind=ck)

    from concourse.tile import add_dep_helper

    with tile.TileContext(nc) as tc:
        import contextlib
        ctx = contextlib.ExitStack()
        with ctx:
            const_p = ctx.enter_context(tc.tile_pool(name="const", bufs=1))
            pre_p = ctx.enter_context(tc.tile_pool(name="pre", bufs=8))
            pre_ps = ctx.enter_context(
                tc.tile_pool(name="preps", bufs=3, space="PSUM")) \
                if stage == 1 else None
            g_p = ctx.enter_context(tc.tile_pool(name="gath", bufs=3))
            e_p = ctx.enter_context(tc.tile_pool(name="edge", bufs=3))
            sel_p = ctx.enter_context(tc.tile_pool(name="sel", bufs=3))
            if stage == 2:
                acc_ps = ctx.enter_context(
                    tc.tile_pool(name="accps", bufs=2, space="PSUM"))
                gbe_ps = ctx.enter_context(
                    tc.tile_pool(name="gbeps", bufs=1, space="PSUM"))
                fin_ps = ctx.enter_context(
                    tc.tile_pool(name="finps", bufs=1, space="PSUM"))
            fin_p = ctx.enter_context(tc.tile_pool(name="fin", bufs=2))

            # ---- constants in SBUF
            ident = const_p.tile([P, P], f32)
            make_identity(nc, ident[:])
            w1_sb = const_p.tile([D, D], f32)
            nc.sync.dma_start(w1_sb[:], t_w1.ap())
            w2_sb = const_p.tile([D, D], f32)
            nc.sync.dma_start(w2_sb[:], t_w2.ap())
            wn_sb = const_p.tile([2 * D, D], f32)
            nc.sync.dma_start(wn_sb[:], t_wn.ap())
            # broadcast rows: b_attn, b_node, w_fc replicated across partitions
            ba_b = const_p.tile([P, D], f32)
            nc.sync.dma_start(ba_b[:], t_ba.ap()[None, :].broadcast_to((P, D)))
            bn_b = const_p.tile([P, D], f32)
            nc.sync.dma_start(bn_b[:], t_bn.ap()[None, :].broadcast_to((P, D)))
            wfc_b = const_p.tile([P, D], f32)
            nc.sync.dma_start(wfc_b[:], t_wfc.ap()[None, :].broadcast_to((P, D)))
            iota_f = const_p.tile([P, P], f32)
            iota_i = const_p.tile([P, P], mybir.dt.int32)
            nc.gpsimd.iota(iota_i[:], pattern=[[1, P]], base=0,
                           channel_multiplier=0)
            nc.vector.tensor_copy(iota_f[:], iota_i[:])
            iota_cf = const_p.tile([P, 1], f32)
            iota_ci = const_p.tile([P, 1], mybir.dt.int32)
            nc.gpsimd.iota(iota_ci[:], pattern=[[1, 1]], base=0,
                           channel_multiplier=1)
            nc.vector.tensor_copy(iota_cf[:], iota_ci[:])
            ones_bps = const_p.tile([P, BPS], bf16)
            nc.vector.memset(ones_bps[:], 1.0)
            dginv_sb = const_p.tile([P, SLOTS], f32)
            nc.sync.dma_start(dginv_sb[:], t_dginv.ap())

            # ---- stage 1: A = nf @ W1 + b_attn ; B = nf_own @ W2
            def proj(dst_dram, src_dram, n_rows, w_sb, bias_b, with_nf):
                nt = n_rows // P
                i = 0
                while i < nt:
                    nb = min(4, nt - i)
                    r0 = i * P
                    x = pre_p.tile([P, nb, D], f32, tag="px")
                    nc.sync.dma_start(
                        x[:], src_dram.ap()[r0:r0 + nb * P, :].rearrange(
                            "(b p) d -> p b d", p=P))
                    xt_ps = pre_ps.tile([D, nb * P], f32, tag="ppt")
                    for j in range(nb):
                        nc.tensor.transpose(
                            out=xt_ps[:, j * P:(j + 1) * P],
                            in_=x[:, j, :], identity=ident[:])
                    xt = pre_p.tile([D, nb * P], f32, tag="pxts")
                    nc.vector.tensor_copy(xt[:], xt_ps[:])
                    y_ps = pre_ps.tile([P, nb, D], f32, tag="ppy")
                    for j in range(nb):
                        nc.tensor.matmul(
                            out=y_ps[:, j, :],
                            lhsT=xt[:, j * P:(j + 1) * P], rhs=w_sb[:],
                            start=True, stop=True)
                    if with_nf:
                        y = pre_p.tile([P, nb, 2 * D], bf16, tag="pyc")
                        nc.vector.tensor_add(
                            y[:, :, :D], y_ps[:],
                            bias_b[:].unsqueeze(1).broadcast_to((P, nb, D)))
                        nc.vector.tensor_copy(y[:, :, D:], x[:])
                        nc.sync.dma_start(
                            dst_dram.ap()[r0:r0 + nb * P, :].rearrange(
                                "(b p) d -> p b d", p=P), y[:])
                    else:
                        y = pre_p.tile([P, nb, D], bf16, tag="pys")
                        nc.vector.tensor_copy(y[:], y_ps[:])
                        nc.sync.dma_start(
                            dst_dram.ap()[r0:r0 + nb * P, :].rearrange(
                                "(b p) d -> p b d", p=P), y[:])
                    i += nb

            if stage == 1:
                proj(t_C, t_nf, NF_PAD, w1_sb, ba_b, True)
                proj(t_B, t_nfo, NPAD, w2_sb, None, False)

            # ---- stage 2: per-slot edge processing
            for s in (range(SLOTS) if stage == 2 else ()):
                i0 = g_p.tile([P, B0 * 8], i16, tag="i0")
                nc.sync.dma_start(i0[:], t_is0.ap()[s])
                i1 = g_p.tile([P, B1 * 8], i16, tag="i1")
                nc.sync.dma_start(i1[:], t_is1.ap()[s])
                pm = g_p.tile([P, BPS], f32, tag="pm")
                nc.sync.dma_start(pm[:], t_pm.ap()[s])
                pmT = g_p.tile([P, BPS * P], f32, tag="pmT")
                nc.sync.dma_start(
                    pmT[:], t_pmT.ap()[s][None, :].broadcast_to((P, BPS * P)))
                B_slot = g_p.tile([P, D], bf16, tag="Bs")
                nc.sync.dma_start(B_slot[:], t_B.ap()[s * P:(s + 1) * P, :])

                gC = g_p.tile([P, BPS, 2 * D], bf16, tag="gC")
                nc.gpsimd.dma_gather(
                    out_ap=gC[:, :B0, :], in_ap=t_C.ap()[:HALF, :],
                    idxs_ap=i0[:], num_idxs=B0 * P, num_idxs_reg=B0 * P,
                    elem_size=2 * D, single_packet=False)
                nc.gpsimd.dma_gather(
                    out_ap=gC[:, B0:, :], in_ap=t_C.ap()[HALF:, :],
                    idxs_ap=i1[:], num_idxs=B1 * P, num_idxs_reg=B1 * P,
                    elem_size=2 * D, single_packet=False)

                # selection matrices, batched: sel_all[e,b,p], selT_all[p,b,e]
                sel_all = sel_p.tile([P, BPS, P], bf16, tag="sela")
                nc.vector.tensor_tensor(
                    out=sel_all[:],
                    in0=pm[:].unsqueeze(2).broadcast_to((P, BPS, P)),
                    in1=iota_f[:].unsqueeze(1).broadcast_to((P, BPS, P)),
                    op=_ALU.is_equal)
                selT_all = sel_p.tile([P, BPS, P], bf16, tag="selTa")
                nc.vector.tensor_tensor(
                    out=selT_all[:],
                    in0=pmT[:].rearrange("p (b e) -> p b e", b=BPS),
                    in1=iota_cf[:].unsqueeze(2).broadcast_to((P, BPS, P)),
                    op=_ALU.is_equal)

                # expand B per-edge: gBe[e,:] = B_slot[pmod_e,:] via PE
                NCH = (BPS + 7) // 8
                gBe_ps = []
                for ch in range(NCH):
                    bpc = min(8, BPS - ch * 8)
                    psb = gbe_ps.tile([P, bpc * D], f32, tag=f"gbe{ch}")
                    gBe_ps.append(psb)
                    for j in range(bpc):
                        b = ch * 8 + j
                        nc.tensor.matmul(
                            out=psb[:, j * D:(j + 1) * D],
                            lhsT=selT_all[:, b, :], rhs=B_slot[:],
                            start=True, stop=True)

                # h = relu(A[src] + B[dst]); logit = h . wfc; lrelu; exp
                h = e_p.tile([P, BPS, D], bf16, tag="h")
                for ch in range(NCH):
                    bpc = min(8, BPS - ch * 8)
                    sl = slice(ch * 8, ch * 8 + bpc)
                    nc.vector.tensor_add(
                        h[:, sl, :], gC[:, sl, :D],
                        gBe_ps[ch][:].rearrange("p (b d) -> p b d", d=D))
                nc.scalar.activation(h[:], h[:], func=_AFT.Relu)
                hw = e_p.tile([P, BPS, D], bf16, tag="hw")
                nc.vector.tensor_mul(
                    hw[:], h[:],
                    wfc_b[:].unsqueeze(1).broadcast_to((P, BPS, D)))
                lg = e_p.tile([P, BPS], f32, tag="lg")
                nc.vector.tensor_reduce(lg[:], hw[:], axis=_ALT.X,
                                        op=_ALU.add)
                lgs = e_p.tile([P, BPS], f32, tag="lgs")
                nc.vector.tensor_scalar_mul(lgs[:], lg[:], 0.01)
                lg2 = e_p.tile([P, BPS], f32, tag="lg2")
                nc.vector.tensor_tensor(out=lg2[:], in0=lg[:], in1=lgs[:],
                                        op=_ALU.max)
                wt = e_p.tile([P, BPS], f32, tag="wt")
                nc.scalar.activation(wt[:], lg2[:], func=_AFT.Exp)

                selw_all = sel_p.tile([P, BPS, P], bf16, tag="selwa")
                nc.vector.tensor_mul(
                    selw_all[:], sel_all[:],
                    wt[:].unsqueeze(2).broadcast_to((P, BPS, P)))

                # payload [nf_bf16 | 1]; one matmul per block accumulates msg+wsum
                pay = e_p.tile([P, BPS, D + 1], bf16, tag="pay")
                nc.vector.tensor_copy(
                    pay[:, :, :D], gC[:, :, D:])
                nc.vector.tensor_copy(pay[:, :, D], ones_bps[:])
                ps = acc_ps.tile([P, D + 1], f32, tag="ps")
                for b in range(BPS):
                    nc.tensor.matmul(out=ps[:], lhsT=selw_all[:, b, :],
                                     rhs=pay[:, b, :],
                                     start=(b == 0), stop=(b == BPS - 1))

                # finalize slot: agg_m = ps[:,:64] / wsum / deg
                den = fin_p.tile([P, 1], f32, tag="den")
                nc.vector.tensor_scalar_max(den[:], ps[:, D:D + 1], 1e-30)
                rec = fin_p.tile([P, 1], f32, tag="rec")
                nc.vector.reciprocal(rec[:], den[:])
                rec2 = fin_p.tile([P, 1], f32, tag="rec2")
                nc.vector.tensor_mul(rec2[:], rec[:], dginv_sb[:, s:s + 1])
                am = fin_p.tile([P, D], f32, tag="am")
                nc.vector.tensor_mul(
                    am[:], ps[:, :D],
                    rec2[:].broadcast_to((P, D)))

                nfo_t = fin_p.tile([P, D], f32, tag="nfo")
                nc.sync.dma_start(nfo_t[:], t_nfo.ap()[s * P:(s + 1) * P, :])
                lhsT = fin_p.tile([2 * D, P], f32, tag="lhsT")
                tp1 = fin_ps.tile([D, P], f32, tag="tp")
                nc.tensor.transpose(out=tp1[:], in_=nfo_t[:], identity=ident[:])
                nc.vector.tensor_copy(lhsT[:D, :], tp1[:])
                tp2 = fin_ps.tile([D, P], f32, tag="tp")
                nc.tensor.transpose(out=tp2[:], in_=am[:], identity=ident[:])
                nc.vector.tensor_copy(lhsT[D:, :], tp2[:])
                o_ps = fin_ps.tile([P, D], f32, tag="ops")
                nc.tensor.matmul(out=o_ps[:], lhsT=lhsT[:], rhs=wn_sb[:],
                                 start=True, stop=True)
                o_sb = fin_p.tile([P, D], f32, tag="osb")
                nc.vector.tensor_add(o_sb[:], o_ps[:], bn_b[:])
                nc.sync.dma_start(t_out.ap()[s * P:(s + 1) * P, :], o_sb[:])

    nc.compile()
    return nc


_AFT = None
_ALT = None
_ALU = None
_PROGRAM_CACHE = {}


def _get_program(cfg_key, cfg):
    global _AFT, _ALT, _ALU
    if _AFT is None:
        from concourse import mybir
        _AFT = mybir.ActivationFunctionType
        _ALT = mybir.AxisListType
        _ALU = mybir.AluOpType
    if cfg_key not in _PROGRAM_CACHE:
        _PROGRAM_CACHE[cfg_key] = (_build_program(cfg, 1),
                                   _build_program(cfg, 2))
    return _PROGRAM_CACHE[cfg_key]


def _prep(nf, src, dst, W_attn, b_attn, w_fc, W_node, b_node, n_cores=8):
    N = nf.shape[0]
    E = src.shape[0]
    NPC = N // n_cores
    NPAD = _ceil_div(NPC, P) * P
    SLOTS = NPAD // P
    HALF = _ceil_div(_ceil_div(N, 2), P) * P
    NF_PAD = 2 * HALF
    assert HALF <= 32768 and N - HALF <= HALF

    src = np.asarray(src).astype(np.int64)
    dst = np.asarray(dst).astype(np.int64)
    nf = np.asarray(nf, dtype=np.float32)

    owner = dst // NPC
    dst_local = dst - owner * NPC
    g = dst_local >> 7
    pmod = dst_local & 127
    half = (src >= HALF).astype(np.int64)
    src_local = src - half * HALF

    key = (owner * SLOTS + g) * 2 + half
    order = np.argsort(key, kind="stable")
    skey = key[order]
    nkeys = n_cores * SLOTS * 2
    bounds = np.searchsorted(skey, np.arange(nkeys + 1))
    cnt = np.diff(bounds).reshape(n_cores, SLOTS, 2)
    B0 = max(1, int(_ceil_div(cnt[:, :, 0].max(), P)))
    B1 = max(1, int(_ceil_div(cnt[:, :, 1].max(), P)))
    BPS = B0 + B1

    p_src = np.zeros((n_cores, SLOTS, BPS * P), np.int16)
    p_dst = np.zeros((n_cores, SLOTS, BPS * P), np.int16)
    p_pm = np.full((n_cores, SLOTS, BPS * P), -1.0, np.float32)
    for c in range(n_cores):
        for s in range(SLOTS):
            for h in (0, 1):
                k = (c * SLOTS + s) * 2 + h
                seg = order[bounds[k]:bounds[k + 1]]
                off = 0 if h == 0 else B0 * P
                n = len(seg)
                p_src[c, s, off:off + n] = src_local[seg]
                p_dst[c, s, off:off + n] = dst_local[seg]
                p_pm[c, s, off:off + n] = pmod[seg]

    deg = np.bincount(dst, minlength=N).astype(np.float32)

    cfg = {"NF_PAD": NF_PAD, "HALF": HALF, "NPAD": NPAD, "B0": B0, "B1": B1}

    nf_pad = np.zeros((NF_PAD, D), np.float32)
    nf_pad[:N] = nf

    in_maps = []
    for c in range(n_cores):
        is0 = _wrap16(p_src[c, :, :B0 * P])
        is1 = _wrap16(p_src[c, :, B0 * P:])
        idt = _wrap16(p_dst[c])
        pm = np.ascontiguousarray(
            p_pm[c].reshape(SLOTS, BPS, P).transpose(0, 2, 1))
        pmT = np.ascontiguousarray(p_pm[c])
        nfo = np.zeros((NPAD, D), np.float32)
        nfo[:NPC] = nf[c * NPC:(c + 1) * NPC]
        degc = np.ones(NPAD, np.float32)
        degc[:NPC] = np.maximum(deg[c * NPC:(c + 1) * NPC], 1.0)
        dginv = np.ascontiguousarray(
            (1.0 / degc).reshape(SLOTS, P).T)
        in_maps.append({
            "nf": nf_pad, "nfo": nfo,
            "w1": np.ascontiguousarray(W_attn[:D]).astype(np.float32),
            "w2": np.ascontiguousarray(W_attn[D:]).astype(np.float32),
            "ba": np.asarray(b_attn, np.float32),
            "wfc": np.asarray(w_fc, np.float32),
            "wn": np.asarray(W_node, np.float32),
            "bn": np.asarray(b_node, np.float32),
            "is0": is0, "is1": is1, "idst": idt, "pm": pm, "pmT": pmT,
            "dginv": dginv,
        })
    return cfg, in_maps, NPC


def kernel(nf, src, dst, W_attn, b_attn, w_fc, W_node, b_node):
    import concourse.bass_utils as bass_utils

    nf = np.asarray(nf)
    cfg, in_maps, NPC = _prep(nf, src, dst, W_attn, b_attn, w_fc,
                              W_node, b_node)
    cfg_key = tuple(sorted(cfg.items()))
    nc1, nc2 = _get_program(cfg_key, cfg)
    res1 = bass_utils.run_bass_kernel_spmd(nc1, in_maps,
                                           core_ids=list(range(8)))
    in_maps2 = [{**m, "C_scr": r["C_scr"], "B_scr": r["B_scr"]}
                for m, r in zip(in_maps, res1.results)]
    res = bass_utils.run_bass_kernel_spmd(nc2, in_maps2,
                                          core_ids=list(range(8)))
    out = np.concatenate([res.results[c]["out"][:NPC] for c in range(8)], axis=0)
    return out.astype(np.float32)
